# revision 25
# baseline (speedup 1.0000x reference)
"""Trainium2 Bass kernel for nn_DiffeqSolver (GNN message passing ODE, RK4).

Contract: kernel(**inputs) takes FULL unsharded numpy inputs (as produced by
reference.setup_inputs()) and returns the FULL output [S, b*N, T, F] fp32.

Strategy (data-parallel over batch, 8 items per core on 8 cores):
  All linear algebra is PE matmuls; per-edge gather/broadcast/type-masking is
  folded into host-precomputed masked selector matrices RSm (static per item),
  and edge->node aggregation is a segmented free-dim reduce over mm followed
  by a small W3 matmul (reduce-then-project). The edge-type one-hot selection
  ("wrong type" columns) produces a constant junk term removed inside PSUM by
  a rank-2 accumulating matmul (cj x cnt).

  Schedule: phase-major RK4 - per step, each of the 4 ODE evals runs for all
  8 items back-to-back so the engines pipeline across independent items.
"""

import os
from contextlib import ExitStack

import numpy as np

try:
    import ml_dtypes
    import concourse.bass as bass
    import concourse.mybir as mybir
    import concourse.tile as tile
    from concourse.bass_utils import run_bass_kernel_spmd
    BF16 = mybir.dt.bfloat16
    F32 = mybir.dt.float32
    F32R = mybir.dt.float32r
    _HAVE_BASS = True
except Exception:                                  # bare env: numpy-only path
    _HAVE_BASS = False

# Problem constants (hardcoded per spec nn_DiffeqSolver_42666205118907)
N_CORES = 8
B = 64              # batch items
IPC = B // N_CORES  # items per core
N = 50              # atoms per item
F = 64              # feature dim
H = 128             # hidden dim
K = 2               # edge types
T = 40              # time steps
NE = N * N          # padded edge grid (incl. diagonal)
CH = [(i * 500, 500) for i in range(5)]   # grid chunks (1 psum bank each)

_bf = ml_dtypes.bfloat16 if _HAVE_BASS else None


def _install_ntff_shim():
    """Register the axon NTFF profile hook if the image's antenv lacks the
    axon_hooks submodule (the boot code degrades silently in that case;
    see trn_agent_boot.trn_boot).  Uses the boot module's own ctypes
    factory against the standard libaxon_pjrt.so path."""
    import sys
    import types
    try:
        import antenv.axon_hooks  # noqa: F401
        return
    except ImportError:
        pass
    try:
        import antenv
        from trn_agent_boot.trn_boot import _ntff_profile_via_ctypes
        hook = _ntff_profile_via_ctypes("/opt/axon/libaxon_pjrt.so")
        mod = types.ModuleType("antenv.axon_hooks")
        box = [hook]
        mod.set_axon_ntff_profile_hook = lambda h: box.__setitem__(0, h)
        mod.get_axon_ntff_profile_hook = lambda: box[0]
        sys.modules["antenv.axon_hooks"] = mod
        antenv.axon_hooks = mod
    except Exception:
        pass


def _build_static(graph, W1, b1, W2, b2, W3, b3):
    """Host-side per-item static data: RSm selector matrices, cj/cnt corr."""
    off = np.ones((N, N)) - np.eye(N)
    recv_idx, send_idx = np.where(off)
    eg = (recv_idx * N + send_idx).astype(np.int64)

    # device-faithful junk constant per k: columns where t_k == 0 contribute
    # hh_j = relu(0 + b1) (fp32), mm_j = relu(bf16(W2).T @ hh_j + b2), and the
    # junk flows through magg/W3.  fp32 host math is close enough (<1e-3 rel).
    cj = np.zeros((K, F), np.float32)
    for k in range(K):
        hh_j = np.maximum(b1[k], 0.0).astype(np.float32)
        mm_j = np.maximum(hh_j @ W2[k] + b2[k], 0.0).astype(np.float32)
        cj[k] = mm_j @ W3

    tks = np.zeros((B, K, NE), np.float32)
    tks[np.arange(B)[:, None], graph, eg[None, :]] = 1.0

    # RSm: [B, 128, K, NE] bf16. rows 0:50 sender-selector, 64:114 receiver
    # (rows 50:64 and 114:128 stay zero -- matmul lhsT spans 128 partitions).
    RSm = np.zeros((B, H, K, NE), np.float32)
    ar = np.arange(B)[:, None]
    for k in range(K):
        mk = (graph == k).astype(np.float32)            # [B, E]
        RSm[ar, send_idx[None, :], k, eg[None, :]] = mk
        RSm[ar, 64 + recv_idx[None, :], k, eg[None, :]] = mk

    # cnt[i, k, r] = N - n_{k,r}: junk column count per receiver segment
    nkr = tks.reshape(B, K, N, N).sum(axis=3)           # [B, K, N]
    cnt = (N - nkr).astype(np.float32)                  # [B, K, N]
    return RSm.astype(_bf), cj, cnt


def _trace_program(dt, nsteps):
    """Straight-line Bass/Tile program for one core: IPC items, nsteps RK4
    steps fully unrolled, phase-major (items interleaved within each eval
    phase so independent work pipelines across the engines)."""
    nc = bass.Bass("TRN2", target_bir_lowering=False, debug=False,
                   enable_asserts=False, num_devices=N_CORES,
                   dynamic_dma_scratch_size=2048)

    NBF = IPC * K * NE + 2 * K * H + F                 # rsm + w1 + w3 bf16 cols
    NWM = K * H + F + F + IPC * N                      # w2, w3, cj, negcnt (f32r)
    NBI = K + K + 1                                    # b1, b2, b3 (f32)
    wmat_d = nc.dram_tensor("wmat", [H, NWM], F32R, kind="ExternalInput").ap()
    bias_d = nc.dram_tensor("bias", [H, NBI], F32, kind="ExternalInput").ap()
    rsmw_d = nc.dram_tensor("rsmw", [H, NBF], BF16, kind="ExternalInput").ap()
    yst_d = nc.dram_tensor("ystate", [F, IPC, N], F32, kind="ExternalInput").ap()
    out_d = nc.dram_tensor("yout", [F, nsteps, IPC, N], F32,
                           kind="ExternalOutput").ap()

    AL = mybir.AluOpType
    AF = mybir.ActivationFunctionType

    with tile.TileContext(nc) as tc, ExitStack() as ctx:
        statics = ctx.enter_context(tc.tile_pool(name="statics", bufs=1))
        state = ctx.enter_context(tc.tile_pool(name="state", bufs=1))
        hhp = ctx.enter_context(tc.tile_pool(name="hhp", bufs=3))   # [H,500]
        mmp = ctx.enter_context(tc.tile_pool(name="mmp", bufs=2))   # [H,2500]x2k
        small = ctx.enter_context(tc.tile_pool(name="small", bufs=2))
        dys = ctx.enter_context(tc.tile_pool(name="dys", bufs=1))
        wps = ctx.enter_context(tc.tile_pool(name="wps", bufs=1, space="PSUM"))
        hps = ctx.enter_context(tc.tile_pool(name="hps", bufs=3, space="PSUM"))
        mps = ctx.enter_context(tc.tile_pool(name="mps", bufs=2, space="PSUM"))
        ups = ctx.enter_context(tc.tile_pool(name="ups", bufs=2, space="PSUM"))

        wmat = statics.tile([H, NWM], F32R, name="wmat")
        biast = statics.tile([H, NBI], F32, name="biast")
        rsmw = statics.tile([H, NBF], BF16, name="rsmw")
        nc.sync.dma_start(out=wmat, in_=wmat_d)
        nc.sync.dma_start(out=biast, in_=bias_d)
        nc.sync.dma_start(out=rsmw, in_=rsmw_d)
        w2s = wmat[:, 0:K * H].rearrange("p (k h) -> p k h", k=K)
        w3s = wmat[:, K * H:K * H + F]
        cjs = wmat[0:K, K * H + F:K * H + 2 * F]           # [2, 64] f32r
        negcnt = wmat[0:K, K * H + 2 * F:K * H + 2 * F + IPC * N].rearrange(
            "p (i n) -> p i n", i=IPC)                     # [2, IPC, 50] f32r
        b1s = biast[:, 0:K]
        b2s = biast[:, K:2 * K]
        b3s = biast[0:F, 2 * K:2 * K + 1]
        rsm_all = rsmw[:, 0:IPC * K * NE].rearrange(
            "p (i k e) -> p i k e", i=IPC, k=K)
        w1all = rsmw[0:F, IPC * K * NE:IPC * K * NE + 2 * K * H]  # [64,512] bf16
        w3bf = rsmw[:, IPC * K * NE + 2 * K * H:]          # [128, 64] bf16
        rsms = [rsm_all[:, i] for i in range(IPC)]

        hist = state.tile([F, nsteps, IPC, N], F32, name="hist")
        ybig = state.tile([F, IPC, N], F32, name="ybig")
        nc.sync.dma_start(out=ybig, in_=yst_d)
        asts = []
        for i in range(IPC):
            a = state.tile([H, K, H], BF16, name=f"ast{i}")
            nc.vector.memset(a, 0.0)
            asts.append(a)

        # simple engine load balancer for relu/copy jobs
        load = {"v": 0.0, "a": 0.0}

        def relu_job(out_ap, in_ap, bias_ap, nelem):
            cv = (120 + nelem) / 0.96 + load["v"]
            ca = (172 + nelem) / 1.2 + load["a"]
            if cv <= ca:
                load["v"] = cv
                nc.vector.tensor_scalar(out_ap, in_ap, bias_ap, 0.0,
                                        op0=AL.add, op1=AL.max)
            else:
                load["a"] = ca
                nc.scalar.activation(out_ap, in_ap, AF.Relu, bias=bias_ap)

        def eval_ode(i, ysrc, dy_out):
            """dy_out[64,50] f32 = ode(ysrc[64,50] f32) for item i."""
            ybf = small.tile([F, N], BF16, tag="ybf")
            nc.vector.tensor_scalar_add(ybf, ysrc, 0.0)   # cast-copy (TS struct)
            load["v"] += (58 + N) / 0.96

            # W1 stage: [50, 512] psum = (ys0|ys1|yr0|yr1), one matmul
            wp = wps.tile([N, 4 * H], F32, tag="wp")
            nc.tensor.matmul(wp, ybf, w1all, start=True, stop=True)
            ast = asts[i]
            nc.vector.tensor_scalar_add(
                ast[0:N], wp[:, 0:2 * H].rearrange("p (k h) -> p k h", k=K), 0.0)
            load["v"] += (120 + 2 * H) / 0.96
            nc.scalar.copy(
                ast[64:64 + N], wp[:, 2 * H:4 * H].rearrange("p (k h) -> p k h", k=K))
            load["a"] += (172 + 2 * H) / 1.2

            maggs = []
            for k in range(K):
                mm = mmp.tile([H, NE], BF16, tag=f"mm{k}")
                for (c0, cl) in CH:
                    hp = hps.tile([H, 500], F32, tag="hp")
                    nc.tensor.matmul(hp[:, 0:cl], ast[:, k, :],
                                     rsms[i][:, k, c0:c0 + cl],
                                     start=True, stop=True)
                    ht = hhp.tile([H, 500], F32R, tag="hh")
                    relu_job(ht[:, 0:cl], hp[:, 0:cl], b1s[:, k:k + 1], cl)
                    mp = mps.tile([H, 500], F32, tag="mp")
                    nc.tensor.matmul(mp[:, 0:cl], w2s[:, k, :], ht[:, 0:cl],
                                     start=True, stop=True)
                    relu_job(mm[:, c0:c0 + cl], mp[:, 0:cl],
                             b2s[:, k:k + 1], cl)
                # segmented reduce: [H, 50 r-segs, 50] bf16 -> [H, 50] bf16
                # (all-2-byte operands enable the DVE 2x/4x perf modes)
                magg = small.tile([H, N], F32R, tag=f"magg{k}")
                with nc.allow_low_precision(reason="bf16 magg; quant err ~4e-5"):
                    nc.vector.tensor_reduce(
                        magg, mm.rearrange("p (r s) -> p r s", r=N),
                        axis=mybir.AxisListType.X, op=AL.add)
                load["v"] += (58 + NE / 4) / 0.96
                maggs.append(magg)

            u = ups.tile([F, N], F32, tag="u")
            nc.tensor.matmul(u, w3s, maggs[0], start=True, stop=False)
            nc.tensor.matmul(u, w3s, maggs[1], start=False, stop=False)
            # subtract junk: u += cj.T @ (-cnt_i)   (rank-2 over K partitions)
            nc.tensor.matmul(u, cjs, negcnt[:, i, :],
                             start=False, stop=True)
            nc.scalar.activation(dy_out, u, AF.Tanh, bias=b3s[:, 0:1],
                                 scale=1.0 / N)
            load["a"] += (172 + N) / 1.2

        def stt(out, in0, scal, in1):
            # out = in0 * scal + in1
            nc.vector.scalar_tensor_tensor(out, in0, float(scal), in1,
                                           op0=AL.mult, op1=AL.add)

        for s in range(nsteps):
            ycur = ybig if s == 0 else hist[:, s - 1]
            dy1 = dys.tile([F, IPC, N], F32, tag="dy1")
            dy2 = dys.tile([F, IPC, N], F32, tag="dy2")
            dy3 = dys.tile([F, IPC, N], F32, tag="dy3")
            dy4 = dys.tile([F, IPC, N], F32, tag="dy4")
            ya = dys.tile([F, IPC, N], F32, tag="ya")
            ac1 = dys.tile([F, IPC, N], F32, tag="ac1")
            ac2 = dys.tile([F, IPC, N], F32, tag="ac2")

            # phase 1: dy1 = ode(y)
            for i in range(IPC):
                eval_ode(i, ycur[:, i, :], dy1[:, i, :])
                stt(ya[:, i, :], dy1[:, i, :], dt / 2, ycur[:, i, :])
            # phase 2: dy2 = ode(y + dt/2 dy1)
            for i in range(IPC):
                eval_ode(i, ya[:, i, :], dy2[:, i, :])
                stt(ya[:, i, :], dy2[:, i, :], dt / 2, ycur[:, i, :])
            stt(ac1, dy2, 2.0, dy1)                      # batched, off-path
            # phase 3: dy3 = ode(y + dt/2 dy2)
            for i in range(IPC):
                eval_ode(i, ya[:, i, :], dy3[:, i, :])
                stt(ya[:, i, :], dy3[:, i, :], float(dt), ycur[:, i, :])
            stt(ac2, dy3, 2.0, ac1)
            # phase 4: dy4 = ode(y + dt dy3); y_new per item
            for i in range(IPC):
                eval_ode(i, ya[:, i, :], dy4[:, i, :])
                ac3 = small.tile([F, N], F32, tag="ac3")
                nc.vector.tensor_tensor(ac3, dy4[:, i, :], ac2[:, i, :],
                                        op=AL.add)
                stt(hist[:, s, i, :], ac3, dt / 6, ycur[:, i, :])

        nc.sync.dma_start(out=out_d, in_=hist)

    # TRN2 codegen allows at most one sync-wait per instruction; these Bacc
    # legalization passes split excess waits onto event-semaphore instrs.
    import bass_rust
    bass_rust.move_matmul_waits_to_ldweights(nc.m)
    bass_rust.generate_event_semaphores(nc)
    return nc


def _kernel_numpy(first_point, dt, graph, W1, b1, W2, b2, W3, b3):
    """Vectorized numpy fallback implementing the exact reference math."""
    off = np.ones((N, N)) - np.eye(N)
    recv_idx, send_idx = np.where(off)      # r-major: 49 consecutive per r
    E = len(recv_idx)
    y = first_point.reshape(B, N, F).astype(np.float32)      # [B, N, F]
    sel0 = (graph == 0)[:, :, None]                          # [B, E, 1]

    W1f = W1.astype(np.float32)   # [K, 2F, H]
    W1s = np.ascontiguousarray(W1f[:, :F].transpose(1, 0, 2).reshape(F, K * H))
    W1r = np.ascontiguousarray(W1f[:, F:].transpose(1, 0, 2).reshape(F, K * H))
    W2a = W2.astype(np.float32)
    inv_n = np.float32(1.0 / N)

    def ode(yb):
        ysd = (yb.reshape(-1, F) @ W1s).reshape(-1, N, K, H)   # [B, N, K, H]
        yrc = (yb.reshape(-1, F) @ W1r).reshape(-1, N, K, H)
        h = ysd[:, send_idx] + yrc[:, recv_idx] + b1[None, None]   # [B, E, K, H]
        np.maximum(h, 0.0, out=h)
        m0 = h[:, :, 0].reshape(-1, H) @ W2a[0] + b2[0]
        m1 = h[:, :, 1].reshape(-1, H) @ W2a[1] + b2[1]
        np.maximum(m0, 0.0, out=m0)
        np.maximum(m1, 0.0, out=m1)
        msel = np.where(sel0, m0.reshape(-1, E, H), m1.reshape(-1, E, H))
        agg = msel.reshape(-1, N, N - 1, H).sum(axis=2)         # [B, N, H]
        return np.tanh((agg * inv_n).reshape(-1, H) @ W3 + b3).reshape(-1, N, F)

    outs = [y.copy()]
    for s in range(T - 1):
        k1 = ode(y)
        k2 = ode(y + (0.5 * dt) * k1)
        k3 = ode(y + (0.5 * dt) * k2)
        k4 = ode(y + dt * k3)
        y = y + (dt / 6.0) * (k1 + 2 * k2 + 2 * k3 + k4)
        outs.append(y.copy())
    pred = np.stack(outs, axis=0)                            # [T, B, N, F]
    return np.ascontiguousarray(
        pred.transpose(1, 2, 0, 3).reshape(1, B * N, T, F).astype(np.float32))


def kernel(first_point, time_steps, graph, W1, b1, W2, b2, W3, b3):
    first_point = np.asarray(first_point, dtype=np.float32)
    time_steps = np.asarray(time_steps, dtype=np.float32)
    graph = np.asarray(graph).astype(np.int64)
    W1 = np.asarray(W1, dtype=np.float32)
    b1 = np.asarray(b1, dtype=np.float32)
    W2 = np.asarray(W2, dtype=np.float32)
    b2 = np.asarray(b2, dtype=np.float32)
    W3 = np.asarray(W3, dtype=np.float32)
    b3 = np.asarray(b3, dtype=np.float32)

    dts = np.diff(time_steps.astype(np.float64))
    assert np.allclose(dts, dts.mean(), rtol=1e-4), "non-uniform dt unsupported"
    dt = float(dts.mean())

    if not _HAVE_BASS or os.environ.get("KFORCE_NUMPY", "0") == "1":
        return _kernel_numpy(first_point, dt, graph, W1, b1, W2, b2, W3, b3)

    RSm, cj, cnt = _build_static(graph, W1, b1, W2, b2, W3, b3)

    # y0 per item, transposed: [B, F, N]
    y0t = np.ascontiguousarray(
        first_point.reshape(B, N, F).transpose(0, 2, 1)).astype(np.float32)

    w1t = np.zeros((H, 2 * K * H), np.float32)
    w1t[0:F] = W1.reshape(K, 2, F, H).transpose(2, 1, 0, 3).reshape(F, 2 * K * H)

    nsteps = int(os.environ.get("KNSTEPS", str(T - 1)))
    n_launch = -(-(T - 1) // nsteps)
    try:
        nc = _trace_program(dt, nsteps)

        NWM = K * H + F + F + IPC * N
        bias_h = np.zeros((H, K + K + 1), np.float32)
        bias_h[:, 0:K] = b1.T
        bias_h[:, K:2 * K] = b2.T
        bias_h[0:F, 2 * K] = b3
        wmats, rsmws = [], []
        for c in range(N_CORES):
            sl = slice(c * IPC, (c + 1) * IPC)
            wm = np.zeros((H, NWM), np.float32)
            wm[:, 0:K * H] = W2.transpose(1, 0, 2).reshape(H, K * H)
            wm[:, K * H:K * H + F] = W3
            wm[0:K, K * H + F:K * H + 2 * F] = cj
            wm[0:K, K * H + 2 * F:K * H + 2 * F + IPC * N] = (
                -cnt[sl].transpose(1, 0, 2).reshape(K, IPC * N))
            wmats.append(np.ascontiguousarray(wm))
            rsmws.append(np.ascontiguousarray(np.concatenate([
                RSm[sl].transpose(1, 0, 2, 3).reshape(H, IPC * K * NE)
                .astype(np.float32), w1t, W3], axis=1).astype(_bf)))

        ystate = [np.ascontiguousarray(y0t[c * IPC:(c + 1) * IPC].transpose(1, 0, 2))
                  for c in range(N_CORES)]
        chunks = []
        kernel.last_results = []
        want_trace = bool(int(os.environ.get("KTRACE", "1")))
        if want_trace:
            _install_ntff_shim()
        for L in range(n_launch):
            in_maps = [{"wmat": wmats[c], "bias": bias_h, "rsmw": rsmws[c],
                        "ystate": ystate[c]}
                       for c in range(N_CORES)]
            try:
                res = run_bass_kernel_spmd(nc, in_maps,
                                           core_ids=list(range(N_CORES)),
                                           trace=want_trace)
            except Exception:
                if not want_trace:
                    raise
                import traceback
                traceback.print_exc()
                print("kernel: traced launch failed; retrying untraced")
                want_trace = False
                res = run_bass_kernel_spmd(nc, in_maps,
                                           core_ids=list(range(N_CORES)),
                                           trace=False)
            kernel.last_results.append(res)
            outs = [r["yout"] for r in res.results]          # [F, ns, IPC, N]
            chunks.append(np.stack(outs, axis=0))            # [C, F, ns, IPC, N]
            ystate = [np.ascontiguousarray(o_[:, -1, :, :]) for o_ in outs]

        allc = np.concatenate(chunks, axis=2)[:, :, :T - 1]  # [C, F, T-1, IPC, N]
        yout = np.transpose(allc, (0, 3, 2, 1, 4)).reshape(B, T - 1, F, N)
        full = np.concatenate([y0t[:, None], yout], axis=1)   # [B, T, F, N]
        pred = np.transpose(full, (0, 3, 1, 2)).reshape(1, B * N, T, F)
        return np.ascontiguousarray(pred.astype(np.float32))
    except Exception as e:
        import traceback
        traceback.print_exc()
        print("kernel: device path failed; numpy fallback", repr(e)[:200])
        return _kernel_numpy(first_point, dt, graph, W1, b1, W2, b2, W3, b3)


if __name__ == "__main__":
    import reference
    inputs = {k: np.asarray(v) for k, v in reference.setup_inputs().items()}
    out = kernel(**inputs)
    print("out", out.shape, out.dtype)


# revision 26
# speedup vs baseline: 1652.0644x; 1652.0644x over previous
"""Trainium2 Bass kernel for nn_DiffeqSolver (GNN message passing ODE, RK4).

Contract: kernel(**inputs) takes FULL unsharded numpy inputs (as produced by
reference.setup_inputs()) and returns the FULL output [S, b*N, T, F] fp32.

Strategy (data-parallel over batch, 8 items per core on 8 cores):
  All linear algebra is PE matmuls; per-edge gather/broadcast/type-masking is
  folded into host-precomputed masked selector matrices RSm (static per item),
  and edge->node aggregation is a segmented free-dim reduce over mm followed
  by a small W3 matmul (reduce-then-project). The edge-type one-hot selection
  ("wrong type" columns) produces a constant junk term removed inside PSUM by
  a rank-2 accumulating matmul (cj x cnt).

  Schedule: phase-major RK4 - per step, each of the 4 ODE evals runs for all
  8 items back-to-back so the engines pipeline across independent items.
"""

import os
from contextlib import ExitStack

import numpy as np

try:
    import ml_dtypes
    import concourse.bass as bass
    import concourse.mybir as mybir
    import concourse.tile as tile
    from concourse.bass_utils import run_bass_kernel_spmd
    BF16 = mybir.dt.bfloat16
    F32 = mybir.dt.float32
    F32R = mybir.dt.float32r
    _HAVE_BASS = True
except Exception:                                  # bare env: numpy-only path
    _HAVE_BASS = False

# Problem constants (hardcoded per spec nn_DiffeqSolver_42666205118907)
N_CORES = 8
B = 64              # batch items
IPC = B // N_CORES  # items per core
N = 50              # atoms per item
F = 64              # feature dim
H = 128             # hidden dim
K = 2               # edge types
T = 40              # time steps
NE = N * N          # padded edge grid (incl. diagonal)
CH = [(i * 500, 500) for i in range(5)]   # grid chunks (1 psum bank each)

_bf = ml_dtypes.bfloat16 if _HAVE_BASS else None


def _install_ntff_shim():
    """Register the axon NTFF profile hook if the image's antenv lacks the
    axon_hooks submodule (the boot code degrades silently in that case;
    see trn_agent_boot.trn_boot).  Uses the boot module's own ctypes
    factory against the standard libaxon_pjrt.so path."""
    import sys
    import types
    try:
        import antenv.axon_hooks  # noqa: F401
        return
    except ImportError:
        pass
    try:
        import antenv
        from trn_agent_boot.trn_boot import _ntff_profile_via_ctypes
        hook = _ntff_profile_via_ctypes("/opt/axon/libaxon_pjrt.so")
        mod = types.ModuleType("antenv.axon_hooks")
        box = [hook]
        mod.set_axon_ntff_profile_hook = lambda h: box.__setitem__(0, h)
        mod.get_axon_ntff_profile_hook = lambda: box[0]
        sys.modules["antenv.axon_hooks"] = mod
        antenv.axon_hooks = mod
    except Exception:
        pass


def _build_static(graph, W1, b1, W2, b2, W3, b3):
    """Host-side per-item static data: RSm selector matrices, cj/cnt corr."""
    off = np.ones((N, N)) - np.eye(N)
    recv_idx, send_idx = np.where(off)
    eg = (recv_idx * N + send_idx).astype(np.int64)

    # device-faithful junk constant per k: columns where t_k == 0 contribute
    # hh_j = relu(0 + b1) (fp32), mm_j = relu(bf16(W2).T @ hh_j + b2), and the
    # junk flows through magg/W3.  fp32 host math is close enough (<1e-3 rel).
    cj = np.zeros((K, F), np.float32)
    for k in range(K):
        hh_j = np.maximum(b1[k], 0.0).astype(np.float32)
        mm_j = np.maximum(hh_j @ W2[k] + b2[k], 0.0).astype(np.float32)
        cj[k] = mm_j @ W3

    tks = np.zeros((B, K, NE), np.float32)
    tks[np.arange(B)[:, None], graph, eg[None, :]] = 1.0

    # RSm: [B, 128, K, NE] bf16. rows 0:50 sender-selector, 64:114 receiver
    # (rows 50:64 and 114:128 stay zero -- matmul lhsT spans 128 partitions).
    RSm = np.zeros((B, H, K, NE), np.float32)
    ar = np.arange(B)[:, None]
    for k in range(K):
        mk = (graph == k).astype(np.float32)            # [B, E]
        RSm[ar, send_idx[None, :], k, eg[None, :]] = mk
        RSm[ar, 64 + recv_idx[None, :], k, eg[None, :]] = mk

    # cnt[i, k, r] = N - n_{k,r}: junk column count per receiver segment
    nkr = tks.reshape(B, K, N, N).sum(axis=3)           # [B, K, N]
    cnt = (N - nkr).astype(np.float32)                  # [B, K, N]
    return RSm.astype(_bf), cj, cnt


def _trace_program(dt, nsteps):
    """Straight-line Bass/Tile program for one core: IPC items, nsteps RK4
    steps fully unrolled, phase-major (items interleaved within each eval
    phase so independent work pipelines across the engines)."""
    nc = bass.Bass("TRN2", target_bir_lowering=False, debug=False,
                   enable_asserts=False, num_devices=N_CORES,
                   dynamic_dma_scratch_size=2048)

    NBF = IPC * K * NE + 2 * K * H + F                 # rsm + w1 + w3 bf16 cols
    NWM = K * H + F + F + IPC * N                      # w2, w3, cj, negcnt (f32r)
    NBI = K + K + 1                                    # b1, b2, b3 (f32)
    wmat_d = nc.dram_tensor("wmat", [H, NWM], F32R, kind="ExternalInput").ap()
    bias_d = nc.dram_tensor("bias", [H, NBI], F32, kind="ExternalInput").ap()
    rsmw_d = nc.dram_tensor("rsmw", [H, NBF], BF16, kind="ExternalInput").ap()
    yst_d = nc.dram_tensor("ystate", [F, IPC, N], F32, kind="ExternalInput").ap()
    out_d = nc.dram_tensor("yout", [F, nsteps, IPC, N], F32,
                           kind="ExternalOutput").ap()

    AL = mybir.AluOpType
    AF = mybir.ActivationFunctionType

    with tile.TileContext(nc) as tc, ExitStack() as ctx:
        statics = ctx.enter_context(tc.tile_pool(name="statics", bufs=1))
        state = ctx.enter_context(tc.tile_pool(name="state", bufs=1))
        hhp = ctx.enter_context(tc.tile_pool(name="hhp", bufs=3))   # [H,500]
        mmp = ctx.enter_context(tc.tile_pool(name="mmp", bufs=2))   # [H,2500]x2k
        small = ctx.enter_context(tc.tile_pool(name="small", bufs=2))
        dys = ctx.enter_context(tc.tile_pool(name="dys", bufs=1))
        wps = ctx.enter_context(tc.tile_pool(name="wps", bufs=1, space="PSUM"))
        hps = ctx.enter_context(tc.tile_pool(name="hps", bufs=3, space="PSUM"))
        mps = ctx.enter_context(tc.tile_pool(name="mps", bufs=2, space="PSUM"))
        ups = ctx.enter_context(tc.tile_pool(name="ups", bufs=2, space="PSUM"))

        wmat = statics.tile([H, NWM], F32R, name="wmat")
        biast = statics.tile([H, NBI], F32, name="biast")
        rsmw = statics.tile([H, NBF], BF16, name="rsmw")
        nc.sync.dma_start(out=wmat, in_=wmat_d)
        nc.sync.dma_start(out=biast, in_=bias_d)
        nc.sync.dma_start(out=rsmw, in_=rsmw_d)
        w2s = wmat[:, 0:K * H].rearrange("p (k h) -> p k h", k=K)
        w3s = wmat[:, K * H:K * H + F]
        cjs = wmat[0:K, K * H + F:K * H + 2 * F]           # [2, 64] f32r
        negcnt = wmat[0:K, K * H + 2 * F:K * H + 2 * F + IPC * N].rearrange(
            "p (i n) -> p i n", i=IPC)                     # [2, IPC, 50] f32r
        b1s = biast[:, 0:K]
        b2s = biast[:, K:2 * K]
        b3s = biast[0:F, 2 * K:2 * K + 1]
        rsm_all = rsmw[:, 0:IPC * K * NE].rearrange(
            "p (i k e) -> p i k e", i=IPC, k=K)
        w1all = rsmw[0:F, IPC * K * NE:IPC * K * NE + 2 * K * H]  # [64,512] bf16
        w3bf = rsmw[:, IPC * K * NE + 2 * K * H:]          # [128, 64] bf16
        rsms = [rsm_all[:, i] for i in range(IPC)]

        hist = state.tile([F, nsteps, IPC, N], F32, name="hist")
        ybig = state.tile([F, IPC, N], F32, name="ybig")
        nc.sync.dma_start(out=ybig, in_=yst_d)
        asts = []
        for i in range(IPC):
            a = state.tile([H, K, H], BF16, name=f"ast{i}")
            nc.vector.memset(a, 0.0)
            asts.append(a)

        # simple engine load balancer for relu/copy jobs
        load = {"v": 0.0, "a": 0.0}

        def relu_job(out_ap, in_ap, bias_ap, nelem):
            cv = (120 + nelem) / 0.96 + load["v"]
            ca = (172 + nelem) / 1.2 + load["a"]
            if cv <= ca:
                load["v"] = cv
                nc.vector.tensor_scalar(out_ap, in_ap, bias_ap, 0.0,
                                        op0=AL.add, op1=AL.max)
            else:
                load["a"] = ca
                nc.scalar.activation(out_ap, in_ap, AF.Relu, bias=bias_ap)

        def eval_ode(i, ysrc, dy_out):
            """dy_out[64,50] f32 = ode(ysrc[64,50] f32) for item i."""
            ybf = small.tile([F, N], BF16, tag="ybf")
            nc.vector.tensor_scalar_add(ybf, ysrc, 0.0)   # cast-copy (TS struct)
            load["v"] += (58 + N) / 0.96

            # W1 stage: [50, 512] psum = (ys0|ys1|yr0|yr1), one matmul
            wp = wps.tile([N, 4 * H], F32, tag="wp")
            nc.tensor.matmul(wp, ybf, w1all, start=True, stop=True)
            ast = asts[i]
            nc.vector.tensor_scalar_add(
                ast[0:N], wp[:, 0:2 * H].rearrange("p (k h) -> p k h", k=K), 0.0)
            load["v"] += (120 + 2 * H) / 0.96
            nc.scalar.copy(
                ast[64:64 + N], wp[:, 2 * H:4 * H].rearrange("p (k h) -> p k h", k=K))
            load["a"] += (172 + 2 * H) / 1.2

            maggs = []
            for k in range(K):
                mm = mmp.tile([H, NE], BF16, tag=f"mm{k}")
                for (c0, cl) in CH:
                    hp = hps.tile([H, 500], F32, tag="hp")
                    nc.tensor.matmul(hp[:, 0:cl], ast[:, k, :],
                                     rsms[i][:, k, c0:c0 + cl],
                                     start=True, stop=True)
                    ht = hhp.tile([H, 500], F32R, tag="hh")
                    relu_job(ht[:, 0:cl], hp[:, 0:cl], b1s[:, k:k + 1], cl)
                    mp = mps.tile([H, 500], F32, tag="mp")
                    nc.tensor.matmul(mp[:, 0:cl], w2s[:, k, :], ht[:, 0:cl],
                                     start=True, stop=True)
                    relu_job(mm[:, c0:c0 + cl], mp[:, 0:cl],
                             b2s[:, k:k + 1], cl)
                # segmented reduce: [H, 50 r-segs, 50] bf16 -> [H, 50] bf16
                # (all-2-byte operands enable the DVE 2x/4x perf modes)
                magg = small.tile([H, N], BF16, tag=f"magg{k}")
                with nc.allow_low_precision(reason="bf16 magg; quant err ~4e-5"):
                    nc.vector.tensor_reduce(
                        magg, mm.rearrange("p (r s) -> p r s", r=N),
                        axis=mybir.AxisListType.X, op=AL.add)
                load["v"] += (58 + NE / 4) / 0.96
                maggs.append(magg)

            u = ups.tile([F, N], F32, tag="u")
            nc.tensor.matmul(u, w3bf, maggs[0], start=True, stop=False)
            nc.tensor.matmul(u, w3bf, maggs[1], start=False, stop=False)
            # subtract junk: u += cj.T @ (-cnt_i)   (rank-2 over K partitions)
            nc.tensor.matmul(u, cjs, negcnt[:, i, :],
                             start=False, stop=True)
            nc.scalar.activation(dy_out, u, AF.Tanh, bias=b3s[:, 0:1],
                                 scale=1.0 / N)
            load["a"] += (172 + N) / 1.2

        def stt(out, in0, scal, in1):
            # out = in0 * scal + in1
            nc.vector.scalar_tensor_tensor(out, in0, float(scal), in1,
                                           op0=AL.mult, op1=AL.add)

        for s in range(nsteps):
            ycur = ybig if s == 0 else hist[:, s - 1]
            dy1 = dys.tile([F, IPC, N], F32, tag="dy1")
            dy2 = dys.tile([F, IPC, N], F32, tag="dy2")
            dy3 = dys.tile([F, IPC, N], F32, tag="dy3")
            dy4 = dys.tile([F, IPC, N], F32, tag="dy4")
            ya = dys.tile([F, IPC, N], F32, tag="ya")
            ac1 = dys.tile([F, IPC, N], F32, tag="ac1")
            ac2 = dys.tile([F, IPC, N], F32, tag="ac2")

            # phase 1: dy1 = ode(y)
            for i in range(IPC):
                eval_ode(i, ycur[:, i, :], dy1[:, i, :])
                stt(ya[:, i, :], dy1[:, i, :], dt / 2, ycur[:, i, :])
            # phase 2: dy2 = ode(y + dt/2 dy1)
            for i in range(IPC):
                eval_ode(i, ya[:, i, :], dy2[:, i, :])
                stt(ya[:, i, :], dy2[:, i, :], dt / 2, ycur[:, i, :])
            stt(ac1, dy2, 2.0, dy1)                      # batched, off-path
            # phase 3: dy3 = ode(y + dt/2 dy2)
            for i in range(IPC):
                eval_ode(i, ya[:, i, :], dy3[:, i, :])
                stt(ya[:, i, :], dy3[:, i, :], float(dt), ycur[:, i, :])
            stt(ac2, dy3, 2.0, ac1)
            # phase 4: dy4 = ode(y + dt dy3); y_new per item
            for i in range(IPC):
                eval_ode(i, ya[:, i, :], dy4[:, i, :])
                ac3 = small.tile([F, N], F32, tag="ac3")
                nc.vector.tensor_tensor(ac3, dy4[:, i, :], ac2[:, i, :],
                                        op=AL.add)
                stt(hist[:, s, i, :], ac3, dt / 6, ycur[:, i, :])

        nc.sync.dma_start(out=out_d, in_=hist)

    # TRN2 codegen allows at most one sync-wait per instruction; these Bacc
    # legalization passes split excess waits onto event-semaphore instrs.
    import bass_rust
    bass_rust.move_matmul_waits_to_ldweights(nc.m)
    bass_rust.generate_event_semaphores(nc)
    return nc


def _kernel_numpy(first_point, dt, graph, W1, b1, W2, b2, W3, b3):
    """Vectorized numpy fallback implementing the exact reference math."""
    off = np.ones((N, N)) - np.eye(N)
    recv_idx, send_idx = np.where(off)      # r-major: 49 consecutive per r
    E = len(recv_idx)
    y = first_point.reshape(B, N, F).astype(np.float32)      # [B, N, F]
    sel0 = (graph == 0)[:, :, None]                          # [B, E, 1]

    W1f = W1.astype(np.float32)   # [K, 2F, H]
    W1s = np.ascontiguousarray(W1f[:, :F].transpose(1, 0, 2).reshape(F, K * H))
    W1r = np.ascontiguousarray(W1f[:, F:].transpose(1, 0, 2).reshape(F, K * H))
    W2a = W2.astype(np.float32)
    inv_n = np.float32(1.0 / N)

    def ode(yb):
        ysd = (yb.reshape(-1, F) @ W1s).reshape(-1, N, K, H)   # [B, N, K, H]
        yrc = (yb.reshape(-1, F) @ W1r).reshape(-1, N, K, H)
        h = ysd[:, send_idx] + yrc[:, recv_idx] + b1[None, None]   # [B, E, K, H]
        np.maximum(h, 0.0, out=h)
        m0 = h[:, :, 0].reshape(-1, H) @ W2a[0] + b2[0]
        m1 = h[:, :, 1].reshape(-1, H) @ W2a[1] + b2[1]
        np.maximum(m0, 0.0, out=m0)
        np.maximum(m1, 0.0, out=m1)
        msel = np.where(sel0, m0.reshape(-1, E, H), m1.reshape(-1, E, H))
        agg = msel.reshape(-1, N, N - 1, H).sum(axis=2)         # [B, N, H]
        return np.tanh((agg * inv_n).reshape(-1, H) @ W3 + b3).reshape(-1, N, F)

    outs = [y.copy()]
    for s in range(T - 1):
        k1 = ode(y)
        k2 = ode(y + (0.5 * dt) * k1)
        k3 = ode(y + (0.5 * dt) * k2)
        k4 = ode(y + dt * k3)
        y = y + (dt / 6.0) * (k1 + 2 * k2 + 2 * k3 + k4)
        outs.append(y.copy())
    pred = np.stack(outs, axis=0)                            # [T, B, N, F]
    return np.ascontiguousarray(
        pred.transpose(1, 2, 0, 3).reshape(1, B * N, T, F).astype(np.float32))


def kernel(first_point, time_steps, graph, W1, b1, W2, b2, W3, b3):
    first_point = np.asarray(first_point, dtype=np.float32)
    time_steps = np.asarray(time_steps, dtype=np.float32)
    graph = np.asarray(graph).astype(np.int64)
    W1 = np.asarray(W1, dtype=np.float32)
    b1 = np.asarray(b1, dtype=np.float32)
    W2 = np.asarray(W2, dtype=np.float32)
    b2 = np.asarray(b2, dtype=np.float32)
    W3 = np.asarray(W3, dtype=np.float32)
    b3 = np.asarray(b3, dtype=np.float32)

    dts = np.diff(time_steps.astype(np.float64))
    assert np.allclose(dts, dts.mean(), rtol=1e-4), "non-uniform dt unsupported"
    dt = float(dts.mean())

    if not _HAVE_BASS or os.environ.get("KFORCE_NUMPY", "0") == "1":
        return _kernel_numpy(first_point, dt, graph, W1, b1, W2, b2, W3, b3)

    RSm, cj, cnt = _build_static(graph, W1, b1, W2, b2, W3, b3)

    # y0 per item, transposed: [B, F, N]
    y0t = np.ascontiguousarray(
        first_point.reshape(B, N, F).transpose(0, 2, 1)).astype(np.float32)

    w1t = np.zeros((H, 2 * K * H), np.float32)
    w1t[0:F] = W1.reshape(K, 2, F, H).transpose(2, 1, 0, 3).reshape(F, 2 * K * H)

    nsteps = int(os.environ.get("KNSTEPS", str(T - 1)))
    n_launch = -(-(T - 1) // nsteps)
    try:
        nc = _trace_program(dt, nsteps)

        NWM = K * H + F + F + IPC * N
        bias_h = np.zeros((H, K + K + 1), np.float32)
        bias_h[:, 0:K] = b1.T
        bias_h[:, K:2 * K] = b2.T
        bias_h[0:F, 2 * K] = b3
        wmats, rsmws = [], []
        for c in range(N_CORES):
            sl = slice(c * IPC, (c + 1) * IPC)
            wm = np.zeros((H, NWM), np.float32)
            wm[:, 0:K * H] = W2.transpose(1, 0, 2).reshape(H, K * H)
            wm[:, K * H:K * H + F] = W3
            wm[0:K, K * H + F:K * H + 2 * F] = cj
            wm[0:K, K * H + 2 * F:K * H + 2 * F + IPC * N] = (
                -cnt[sl].transpose(1, 0, 2).reshape(K, IPC * N))
            wmats.append(np.ascontiguousarray(wm))
            rsmws.append(np.ascontiguousarray(np.concatenate([
                RSm[sl].transpose(1, 0, 2, 3).reshape(H, IPC * K * NE)
                .astype(np.float32), w1t, W3], axis=1).astype(_bf)))

        ystate = [np.ascontiguousarray(y0t[c * IPC:(c + 1) * IPC].transpose(1, 0, 2))
                  for c in range(N_CORES)]
        chunks = []
        kernel.last_results = []
        want_trace = bool(int(os.environ.get("KTRACE", "1")))
        if want_trace:
            _install_ntff_shim()
        for L in range(n_launch):
            in_maps = [{"wmat": wmats[c], "bias": bias_h, "rsmw": rsmws[c],
                        "ystate": ystate[c]}
                       for c in range(N_CORES)]
            try:
                res = run_bass_kernel_spmd(nc, in_maps,
                                           core_ids=list(range(N_CORES)),
                                           trace=want_trace)
            except Exception:
                if not want_trace:
                    raise
                import traceback
                traceback.print_exc()
                print("kernel: traced launch failed; retrying untraced")
                want_trace = False
                res = run_bass_kernel_spmd(nc, in_maps,
                                           core_ids=list(range(N_CORES)),
                                           trace=False)
            kernel.last_results.append(res)
            outs = [r["yout"] for r in res.results]          # [F, ns, IPC, N]
            chunks.append(np.stack(outs, axis=0))            # [C, F, ns, IPC, N]
            ystate = [np.ascontiguousarray(o_[:, -1, :, :]) for o_ in outs]

        allc = np.concatenate(chunks, axis=2)[:, :, :T - 1]  # [C, F, T-1, IPC, N]
        yout = np.transpose(allc, (0, 3, 2, 1, 4)).reshape(B, T - 1, F, N)
        full = np.concatenate([y0t[:, None], yout], axis=1)   # [B, T, F, N]
        pred = np.transpose(full, (0, 3, 1, 2)).reshape(1, B * N, T, F)
        return np.ascontiguousarray(pred.astype(np.float32))
    except Exception as e:
        import traceback
        traceback.print_exc()
        print("kernel: device path failed; numpy fallback", repr(e)[:200])
        return _kernel_numpy(first_point, dt, graph, W1, b1, W2, b2, W3, b3)


if __name__ == "__main__":
    import reference
    inputs = {k: np.asarray(v) for k, v in reference.setup_inputs().items()}
    out = kernel(**inputs)
    print("out", out.shape, out.dtype)


# revision 29
# speedup vs baseline: 1684.3756x; 1.0196x over previous
"""Trainium2 Bass kernel for nn_DiffeqSolver (GNN message passing ODE, RK4).

Contract: kernel(**inputs) takes FULL unsharded numpy inputs (as produced by
reference.setup_inputs()) and returns the FULL output [S, b*N, T, F] fp32.

Strategy (data-parallel over batch, 8 items per core on 8 cores):
  All linear algebra is PE matmuls; per-edge gather/broadcast/type-masking is
  folded into host-precomputed masked selector matrices RSm (static per item),
  and edge->node aggregation is a segmented free-dim reduce over mm followed
  by a small W3 matmul (reduce-then-project). The edge-type one-hot selection
  ("wrong type" columns) produces a constant junk term removed inside PSUM by
  a rank-2 accumulating matmul (cj x cnt).

  Schedule: phase-major RK4 - per step, each of the 4 ODE evals runs for all
  8 items back-to-back so the engines pipeline across independent items.
"""

import os
from contextlib import ExitStack

import numpy as np

try:
    import ml_dtypes
    import concourse.bass as bass
    import concourse.mybir as mybir
    import concourse.tile as tile
    from concourse.bass_utils import run_bass_kernel_spmd
    BF16 = mybir.dt.bfloat16
    F32 = mybir.dt.float32
    F32R = mybir.dt.float32r
    _HAVE_BASS = True
except Exception:                                  # bare env: numpy-only path
    _HAVE_BASS = False

# Problem constants (hardcoded per spec nn_DiffeqSolver_42666205118907)
N_CORES = 8
B = 64              # batch items
IPC = B // N_CORES  # items per core
N = 50              # atoms per item
F = 64              # feature dim
H = 128             # hidden dim
K = 2               # edge types
T = 40              # time steps
NE = N * N          # padded edge grid (incl. diagonal)
CH = [(i * 500, 500) for i in range(5)]   # grid chunks (1 psum bank each)

_bf = ml_dtypes.bfloat16 if _HAVE_BASS else None


def _install_ntff_shim():
    """Register the axon NTFF profile hook if the image's antenv lacks the
    axon_hooks submodule (the boot code degrades silently in that case;
    see trn_agent_boot.trn_boot).  Uses the boot module's own ctypes
    factory against the standard libaxon_pjrt.so path."""
    import sys
    import types
    try:
        import antenv.axon_hooks  # noqa: F401
        return
    except ImportError:
        pass
    try:
        import antenv
        from trn_agent_boot.trn_boot import _ntff_profile_via_ctypes
        hook = _ntff_profile_via_ctypes("/opt/axon/libaxon_pjrt.so")
        mod = types.ModuleType("antenv.axon_hooks")
        box = [hook]
        mod.set_axon_ntff_profile_hook = lambda h: box.__setitem__(0, h)
        mod.get_axon_ntff_profile_hook = lambda: box[0]
        sys.modules["antenv.axon_hooks"] = mod
        antenv.axon_hooks = mod
    except Exception:
        pass


def _build_static(graph, W1, b1, W2, b2, W3, b3):
    """Host-side per-item static data: RSm selector matrices, cj/cnt corr."""
    off = np.ones((N, N)) - np.eye(N)
    recv_idx, send_idx = np.where(off)
    eg = (recv_idx * N + send_idx).astype(np.int64)

    # device-faithful junk constant per k: columns where t_k == 0 contribute
    # hh_j = relu(0 + b1) (fp32), mm_j = relu(bf16(W2).T @ hh_j + b2), and the
    # junk flows through magg/W3.  fp32 host math is close enough (<1e-3 rel).
    cj = np.zeros((K, F), np.float32)
    for k in range(K):
        hh_j = np.maximum(b1[k], 0.0).astype(np.float32)
        mm_j = np.maximum(hh_j @ W2[k] + b2[k], 0.0).astype(np.float32)
        cj[k] = mm_j @ W3

    tks = np.zeros((B, K, NE), np.float32)
    tks[np.arange(B)[:, None], graph, eg[None, :]] = 1.0

    # RSm: [B, 128, K, NE] bf16. rows 0:50 sender-selector, 64:114 receiver
    # (rows 50:64 and 114:128 stay zero -- matmul lhsT spans 128 partitions).
    RSm = np.zeros((B, H, K, NE), np.float32)
    ar = np.arange(B)[:, None]
    for k in range(K):
        mk = (graph == k).astype(np.float32)            # [B, E]
        RSm[ar, send_idx[None, :], k, eg[None, :]] = mk
        RSm[ar, 64 + recv_idx[None, :], k, eg[None, :]] = mk

    # cnt[i, k, r] = N - n_{k,r}: junk column count per receiver segment
    nkr = tks.reshape(B, K, N, N).sum(axis=3)           # [B, K, N]
    cnt = (N - nkr).astype(np.float32)                  # [B, K, N]
    return RSm.astype(_bf), cj, cnt


def _trace_program(dt, nsteps):
    """Straight-line Bass/Tile program for one core: IPC items, nsteps RK4
    steps fully unrolled, phase-major (items interleaved within each eval
    phase so independent work pipelines across the engines)."""
    nc = bass.Bass("TRN2", target_bir_lowering=False, debug=False,
                   enable_asserts=False, num_devices=N_CORES,
                   dynamic_dma_scratch_size=2048)

    NBF = IPC * K * NE + 2 * K * H + F                 # rsm + w1 + w3 bf16 cols
    NWM = K * H + F + F + IPC * N                      # w2, w3, cj, negcnt (f32r)
    NBI = K + K + 1                                    # b1, b2, b3 (f32)
    wmat_d = nc.dram_tensor("wmat", [H, NWM], F32R, kind="ExternalInput").ap()
    bias_d = nc.dram_tensor("bias", [H, NBI], F32, kind="ExternalInput").ap()
    rsmw_d = nc.dram_tensor("rsmw", [H, NBF], BF16, kind="ExternalInput").ap()
    yst_d = nc.dram_tensor("ystate", [F, IPC, N], F32, kind="ExternalInput").ap()
    out_d = nc.dram_tensor("yout", [F, nsteps, IPC, N], F32,
                           kind="ExternalOutput").ap()

    AL = mybir.AluOpType
    AF = mybir.ActivationFunctionType

    with tile.TileContext(nc) as tc, ExitStack() as ctx:
        statics = ctx.enter_context(tc.tile_pool(name="statics", bufs=1))
        state = ctx.enter_context(tc.tile_pool(name="state", bufs=1))
        hhp = ctx.enter_context(tc.tile_pool(name="hhp", bufs=3))   # [H,500]
        mmp = ctx.enter_context(tc.tile_pool(name="mmp", bufs=2))   # [H,2500]x2k
        small = ctx.enter_context(tc.tile_pool(name="small", bufs=2))
        dys = ctx.enter_context(tc.tile_pool(name="dys", bufs=1))
        wps = ctx.enter_context(tc.tile_pool(name="wps", bufs=1, space="PSUM"))
        hps = ctx.enter_context(tc.tile_pool(name="hps", bufs=4, space="PSUM"))
        mps = ctx.enter_context(tc.tile_pool(name="mps", bufs=2, space="PSUM"))
        ups = ctx.enter_context(tc.tile_pool(name="ups", bufs=1, space="PSUM"))

        wmat = statics.tile([H, NWM], F32R, name="wmat")
        biast = statics.tile([H, NBI], F32, name="biast")
        rsmw = statics.tile([H, NBF], BF16, name="rsmw")
        nc.sync.dma_start(out=wmat, in_=wmat_d)
        nc.sync.dma_start(out=biast, in_=bias_d)
        nc.sync.dma_start(out=rsmw, in_=rsmw_d)
        w2s = wmat[:, 0:K * H].rearrange("p (k h) -> p k h", k=K)
        w3s = wmat[:, K * H:K * H + F]
        cjs = wmat[0:K, K * H + F:K * H + 2 * F]           # [2, 64] f32r
        negcnt = wmat[0:K, K * H + 2 * F:K * H + 2 * F + IPC * N].rearrange(
            "p (i n) -> p i n", i=IPC)                     # [2, IPC, 50] f32r
        b1s = biast[:, 0:K]
        b2s = biast[:, K:2 * K]
        b3s = biast[0:F, 2 * K:2 * K + 1]
        rsm_all = rsmw[:, 0:IPC * K * NE].rearrange(
            "p (i k e) -> p i k e", i=IPC, k=K)
        w1all = rsmw[0:F, IPC * K * NE:IPC * K * NE + 2 * K * H]  # [64,512] bf16
        w3bf = rsmw[:, IPC * K * NE + 2 * K * H:]          # [128, 64] bf16
        rsms = [rsm_all[:, i] for i in range(IPC)]

        hist = state.tile([F, nsteps, IPC, N], F32, name="hist")
        ybig = state.tile([F, IPC, N], F32, name="ybig")
        nc.sync.dma_start(out=ybig, in_=yst_d)
        asts = []
        for i in range(IPC):
            a = state.tile([H, K, H], BF16, name=f"ast{i}")
            nc.vector.memset(a, 0.0)
            asts.append(a)

        # simple engine load balancer for relu/copy jobs
        load = {"v": 0.0, "a": 0.0}

        def relu_job(out_ap, in_ap, bias_ap, nelem):
            # costs calibrated from ntff profile (both ~640ns @ 500 elems)
            cv = (120 + nelem) / 0.96 + load["v"]
            ca = (260 + nelem) / 1.2 + load["a"]
            if cv <= ca:
                load["v"] = cv
                nc.vector.tensor_scalar(out_ap, in_ap, bias_ap, 0.0,
                                        op0=AL.add, op1=AL.max)
            else:
                load["a"] = ca
                nc.scalar.activation(out_ap, in_ap, AF.Relu, bias=bias_ap)

        def eval_ode(i, ysrc, dy_out):
            """dy_out[64,50] f32 = ode(ysrc[64,50] f32) for item i."""
            ybf = small.tile([F, N], BF16, tag="ybf")
            nc.vector.tensor_scalar_add(ybf, ysrc, 0.0)   # cast-copy (TS struct)
            load["v"] += (58 + N) / 0.96

            # W1 stage: [50, 512] psum = (ys0|ys1|yr0|yr1), one matmul
            wp = wps.tile([N, 4 * H], F32, tag="wp")
            nc.tensor.matmul(wp, ybf, w1all, start=True, stop=True)
            ast = asts[i]
            nc.vector.tensor_scalar_add(
                ast[0:N], wp[:, 0:2 * H].rearrange("p (k h) -> p k h", k=K), 0.0)
            load["v"] += (120 + 2 * H) / 0.96
            nc.scalar.copy(
                ast[64:64 + N], wp[:, 2 * H:4 * H].rearrange("p (k h) -> p k h", k=K))
            load["a"] += (172 + 2 * H) / 1.2

            maggs = []
            for k in range(K):
                mm = mmp.tile([H, NE], BF16, tag=f"mm{k}")
                # h-matmuls run (hps bufs - 1) chunks ahead of m-matmuls so
                # the PE never sits in-queue behind an m-matmul whose hh relu
                # hasn't landed yet.
                hts = {}

                def do_h(ci):
                    c0, cl = CH[ci]
                    hp = hps.tile([H, 500], F32, tag="hp")
                    nc.tensor.matmul(hp[:, 0:cl], ast[:, k, :],
                                     rsms[i][:, k, c0:c0 + cl],
                                     start=True, stop=True)
                    ht = hhp.tile([H, 500], F32R, tag="hh")
                    relu_job(ht[:, 0:cl], hp[:, 0:cl], b1s[:, k:k + 1], cl)
                    hts[ci] = ht

                for ci in range(3):
                    do_h(ci)
                for ci in range(len(CH)):
                    c0, cl = CH[ci]
                    mp = mps.tile([H, 500], F32, tag="mp")
                    nc.tensor.matmul(mp[:, 0:cl], w2s[:, k, :],
                                     hts.pop(ci)[:, 0:cl],
                                     start=True, stop=True)
                    if ci + 3 < len(CH):
                        do_h(ci + 3)
                    relu_job(mm[:, c0:c0 + cl], mp[:, 0:cl],
                             b2s[:, k:k + 1], cl)
                # segmented reduce: [H, 50 r-segs, 50] bf16 -> [H, 50] bf16
                magg = small.tile([H, N], BF16, tag=f"magg{k}")
                with nc.allow_low_precision(reason="bf16 magg; quant err ~4e-5"):
                    nc.vector.tensor_reduce(
                        magg, mm.rearrange("p (r s) -> p r s", r=N),
                        axis=mybir.AxisListType.X, op=AL.add)
                load["v"] += (58 + NE) / 0.96
                maggs.append(magg)

            u = ups.tile([F, N], F32, tag="u")
            nc.tensor.matmul(u, w3bf, maggs[0], start=True, stop=False)
            nc.tensor.matmul(u, w3bf, maggs[1], start=False, stop=False)
            # subtract junk: u += cj.T @ (-cnt_i)   (rank-2 over K partitions)
            nc.tensor.matmul(u, cjs, negcnt[:, i, :],
                             start=False, stop=True)
            nc.scalar.activation(dy_out, u, AF.Tanh, bias=b3s[:, 0:1],
                                 scale=1.0 / N)
            load["a"] += (172 + N) / 1.2

        def stt(out, in0, scal, in1):
            # out = in0 * scal + in1
            nc.vector.scalar_tensor_tensor(out, in0, float(scal), in1,
                                           op0=AL.mult, op1=AL.add)

        for s in range(nsteps):
            ycur = ybig if s == 0 else hist[:, s - 1]
            dy1 = dys.tile([F, IPC, N], F32, tag="dy1")
            dy2 = dys.tile([F, IPC, N], F32, tag="dy2")
            dy3 = dys.tile([F, IPC, N], F32, tag="dy3")
            dy4 = dys.tile([F, IPC, N], F32, tag="dy4")
            ya = dys.tile([F, IPC, N], F32, tag="ya")
            ac1 = dys.tile([F, IPC, N], F32, tag="ac1")
            ac2 = dys.tile([F, IPC, N], F32, tag="ac2")

            # phase 1: dy1 = ode(y)
            for i in range(IPC):
                eval_ode(i, ycur[:, i, :], dy1[:, i, :])
                stt(ya[:, i, :], dy1[:, i, :], dt / 2, ycur[:, i, :])
            # phase 2: dy2 = ode(y + dt/2 dy1)
            for i in range(IPC):
                eval_ode(i, ya[:, i, :], dy2[:, i, :])
                stt(ya[:, i, :], dy2[:, i, :], dt / 2, ycur[:, i, :])
            stt(ac1, dy2, 2.0, dy1)                      # batched, off-path
            # phase 3: dy3 = ode(y + dt/2 dy2)
            for i in range(IPC):
                eval_ode(i, ya[:, i, :], dy3[:, i, :])
                stt(ya[:, i, :], dy3[:, i, :], float(dt), ycur[:, i, :])
            stt(ac2, dy3, 2.0, ac1)
            # phase 4: dy4 = ode(y + dt dy3); y_new per item
            for i in range(IPC):
                eval_ode(i, ya[:, i, :], dy4[:, i, :])
                ac3 = small.tile([F, N], F32, tag="ac3")
                nc.vector.tensor_tensor(ac3, dy4[:, i, :], ac2[:, i, :],
                                        op=AL.add)
                stt(hist[:, s, i, :], ac3, dt / 6, ycur[:, i, :])

        nc.sync.dma_start(out=out_d, in_=hist)

    # TRN2 codegen allows at most one sync-wait per instruction; these Bacc
    # legalization passes split excess waits onto event-semaphore instrs.
    import bass_rust
    bass_rust.move_matmul_waits_to_ldweights(nc.m)
    bass_rust.generate_event_semaphores(nc)
    return nc


def _kernel_numpy(first_point, dt, graph, W1, b1, W2, b2, W3, b3):
    """Vectorized numpy fallback implementing the exact reference math."""
    off = np.ones((N, N)) - np.eye(N)
    recv_idx, send_idx = np.where(off)      # r-major: 49 consecutive per r
    E = len(recv_idx)
    y = first_point.reshape(B, N, F).astype(np.float32)      # [B, N, F]
    sel0 = (graph == 0)[:, :, None]                          # [B, E, 1]

    W1f = W1.astype(np.float32)   # [K, 2F, H]
    W1s = np.ascontiguousarray(W1f[:, :F].transpose(1, 0, 2).reshape(F, K * H))
    W1r = np.ascontiguousarray(W1f[:, F:].transpose(1, 0, 2).reshape(F, K * H))
    W2a = W2.astype(np.float32)
    inv_n = np.float32(1.0 / N)

    def ode(yb):
        ysd = (yb.reshape(-1, F) @ W1s).reshape(-1, N, K, H)   # [B, N, K, H]
        yrc = (yb.reshape(-1, F) @ W1r).reshape(-1, N, K, H)
        h = ysd[:, send_idx] + yrc[:, recv_idx] + b1[None, None]   # [B, E, K, H]
        np.maximum(h, 0.0, out=h)
        m0 = h[:, :, 0].reshape(-1, H) @ W2a[0] + b2[0]
        m1 = h[:, :, 1].reshape(-1, H) @ W2a[1] + b2[1]
        np.maximum(m0, 0.0, out=m0)
        np.maximum(m1, 0.0, out=m1)
        msel = np.where(sel0, m0.reshape(-1, E, H), m1.reshape(-1, E, H))
        agg = msel.reshape(-1, N, N - 1, H).sum(axis=2)         # [B, N, H]
        return np.tanh((agg * inv_n).reshape(-1, H) @ W3 + b3).reshape(-1, N, F)

    outs = [y.copy()]
    for s in range(T - 1):
        k1 = ode(y)
        k2 = ode(y + (0.5 * dt) * k1)
        k3 = ode(y + (0.5 * dt) * k2)
        k4 = ode(y + dt * k3)
        y = y + (dt / 6.0) * (k1 + 2 * k2 + 2 * k3 + k4)
        outs.append(y.copy())
    pred = np.stack(outs, axis=0)                            # [T, B, N, F]
    return np.ascontiguousarray(
        pred.transpose(1, 2, 0, 3).reshape(1, B * N, T, F).astype(np.float32))


def kernel(first_point, time_steps, graph, W1, b1, W2, b2, W3, b3):
    first_point = np.asarray(first_point, dtype=np.float32)
    time_steps = np.asarray(time_steps, dtype=np.float32)
    graph = np.asarray(graph).astype(np.int64)
    W1 = np.asarray(W1, dtype=np.float32)
    b1 = np.asarray(b1, dtype=np.float32)
    W2 = np.asarray(W2, dtype=np.float32)
    b2 = np.asarray(b2, dtype=np.float32)
    W3 = np.asarray(W3, dtype=np.float32)
    b3 = np.asarray(b3, dtype=np.float32)

    dts = np.diff(time_steps.astype(np.float64))
    assert np.allclose(dts, dts.mean(), rtol=1e-4), "non-uniform dt unsupported"
    dt = float(dts.mean())

    if not _HAVE_BASS or os.environ.get("KFORCE_NUMPY", "0") == "1":
        return _kernel_numpy(first_point, dt, graph, W1, b1, W2, b2, W3, b3)

    RSm, cj, cnt = _build_static(graph, W1, b1, W2, b2, W3, b3)

    # y0 per item, transposed: [B, F, N]
    y0t = np.ascontiguousarray(
        first_point.reshape(B, N, F).transpose(0, 2, 1)).astype(np.float32)

    w1t = np.zeros((H, 2 * K * H), np.float32)
    w1t[0:F] = W1.reshape(K, 2, F, H).transpose(2, 1, 0, 3).reshape(F, 2 * K * H)

    nsteps = int(os.environ.get("KNSTEPS", str(T - 1)))
    n_launch = -(-(T - 1) // nsteps)
    try:
        nc = _trace_program(dt, nsteps)

        NWM = K * H + F + F + IPC * N
        bias_h = np.zeros((H, K + K + 1), np.float32)
        bias_h[:, 0:K] = b1.T
        bias_h[:, K:2 * K] = b2.T
        bias_h[0:F, 2 * K] = b3
        wmats, rsmws = [], []
        for c in range(N_CORES):
            sl = slice(c * IPC, (c + 1) * IPC)
            wm = np.zeros((H, NWM), np.float32)
            wm[:, 0:K * H] = W2.transpose(1, 0, 2).reshape(H, K * H)
            wm[:, K * H:K * H + F] = W3
            wm[0:K, K * H + F:K * H + 2 * F] = cj
            wm[0:K, K * H + 2 * F:K * H + 2 * F + IPC * N] = (
                -cnt[sl].transpose(1, 0, 2).reshape(K, IPC * N))
            wmats.append(np.ascontiguousarray(wm))
            rsmws.append(np.ascontiguousarray(np.concatenate([
                RSm[sl].transpose(1, 0, 2, 3).reshape(H, IPC * K * NE)
                .astype(np.float32), w1t, W3], axis=1).astype(_bf)))

        ystate = [np.ascontiguousarray(y0t[c * IPC:(c + 1) * IPC].transpose(1, 0, 2))
                  for c in range(N_CORES)]
        chunks = []
        kernel.last_results = []
        want_trace = bool(int(os.environ.get("KTRACE", "1")))
        if want_trace:
            _install_ntff_shim()
        for L in range(n_launch):
            in_maps = [{"wmat": wmats[c], "bias": bias_h, "rsmw": rsmws[c],
                        "ystate": ystate[c]}
                       for c in range(N_CORES)]
            try:
                res = run_bass_kernel_spmd(nc, in_maps,
                                           core_ids=list(range(N_CORES)),
                                           trace=want_trace)
            except Exception:
                if not want_trace:
                    raise
                import traceback
                traceback.print_exc()
                print("kernel: traced launch failed; retrying untraced")
                want_trace = False
                res = run_bass_kernel_spmd(nc, in_maps,
                                           core_ids=list(range(N_CORES)),
                                           trace=False)
            kernel.last_results.append(res)
            outs = [r["yout"] for r in res.results]          # [F, ns, IPC, N]
            chunks.append(np.stack(outs, axis=0))            # [C, F, ns, IPC, N]
            ystate = [np.ascontiguousarray(o_[:, -1, :, :]) for o_ in outs]

        allc = np.concatenate(chunks, axis=2)[:, :, :T - 1]  # [C, F, T-1, IPC, N]
        yout = np.transpose(allc, (0, 3, 2, 1, 4)).reshape(B, T - 1, F, N)
        full = np.concatenate([y0t[:, None], yout], axis=1)   # [B, T, F, N]
        pred = np.transpose(full, (0, 3, 1, 2)).reshape(1, B * N, T, F)
        return np.ascontiguousarray(pred.astype(np.float32))
    except Exception as e:
        import traceback
        traceback.print_exc()
        print("kernel: device path failed; numpy fallback", repr(e)[:200])
        return _kernel_numpy(first_point, dt, graph, W1, b1, W2, b2, W3, b3)


if __name__ == "__main__":
    import reference
    inputs = {k: np.asarray(v) for k, v in reference.setup_inputs().items()}
    out = kernel(**inputs)
    print("out", out.shape, out.dtype)


# revision 37
# speedup vs baseline: 2267.3128x; 1.3461x over previous
"""Trainium2 Bass kernel for nn_DiffeqSolver (GNN message passing ODE, RK4).

Contract: kernel(**inputs) takes FULL unsharded numpy inputs (as produced by
reference.setup_inputs()) and returns the FULL output [S, b*N, T, F] fp32.

Strategy (data-parallel over batch, 8 items per core on 8 cores):
  All linear algebra is PE matmuls; per-edge gather/broadcast/type-masking is
  folded into host-precomputed masked selector matrices RSm (static per item),
  and edge->node aggregation is a segmented free-dim reduce over mm followed
  by a small W3 matmul (reduce-then-project). The edge-type one-hot selection
  ("wrong type" columns) produces a constant junk term removed inside PSUM by
  a rank-2 accumulating matmul (cj x cnt).

  Schedule: phase-major RK4 - per step, each of the 4 ODE evals runs for all
  8 items back-to-back so the engines pipeline across independent items.
"""

import os
from contextlib import ExitStack

import numpy as np

try:
    import ml_dtypes
    import concourse.bass as bass
    import concourse.mybir as mybir
    import concourse.tile as tile
    from concourse.bass_utils import run_bass_kernel_spmd
    BF16 = mybir.dt.bfloat16
    F32 = mybir.dt.float32
    F32R = mybir.dt.float32r
    _HAVE_BASS = True
except Exception:                                  # bare env: numpy-only path
    _HAVE_BASS = False

# Problem constants (hardcoded per spec nn_DiffeqSolver_42666205118907)
N_CORES = 8
B = 64              # batch items
IPC = B // N_CORES  # items per core
N = 50              # atoms per item
F = 64              # feature dim
H = 128             # hidden dim
K = 2               # edge types
T = 40              # time steps
NE = N * N          # padded edge grid (incl. diagonal)
CH = [(i * 500, 500) for i in range(5)]   # grid chunks (1 psum bank each)

_bf = ml_dtypes.bfloat16 if _HAVE_BASS else None


def _install_ntff_shim():
    """Register the axon NTFF profile hook if the image's antenv lacks the
    axon_hooks submodule (the boot code degrades silently in that case;
    see trn_agent_boot.trn_boot).  Uses the boot module's own ctypes
    factory against the standard libaxon_pjrt.so path."""
    import sys
    import types
    try:
        import antenv.axon_hooks  # noqa: F401
        return
    except ImportError:
        pass
    try:
        import antenv
        from trn_agent_boot.trn_boot import _ntff_profile_via_ctypes
        hook = _ntff_profile_via_ctypes("/opt/axon/libaxon_pjrt.so")
        mod = types.ModuleType("antenv.axon_hooks")
        box = [hook]
        mod.set_axon_ntff_profile_hook = lambda h: box.__setitem__(0, h)
        mod.get_axon_ntff_profile_hook = lambda: box[0]
        sys.modules["antenv.axon_hooks"] = mod
        antenv.axon_hooks = mod
    except Exception:
        pass


def _build_static(graph, W1, b1, W2, b2, W3, b3):
    """Host-side per-item static data: type-packed selector matrices SelP,
    junk constants cj, pad counts cnt, and the global segment widths P[k].

    Column layout per item: block k (k=0,1), 50 receiver segments of fixed
    width P[k]; segment (k, r) holds the senders s of the type-k edges
    (r, s), zero-padded to P[k].  Zero columns produce the constant junk
    relu(b1_k) -> relu(b2_k)-path value, removed by the cj x cnt matmul."""
    off = np.ones((N, N)) - np.eye(N)
    recv_idx, send_idx = np.where(off)

    cj = np.zeros((K, F), np.float32)
    for k in range(K):
        hh_j = np.maximum(b1[k], 0.0).astype(np.float32)
        mm_j = np.maximum(hh_j @ W2[k] + b2[k], 0.0).astype(np.float32)
        cj[k] = mm_j @ W3

    # senders per (item, k, receiver)
    send_lists = [[[[] for _ in range(N)] for _ in range(K)] for _ in range(B)]
    for i in range(B):
        gi = graph[i]
        for e in range(len(recv_idx)):
            send_lists[i][gi[e]][recv_idx[e]].append(send_idx[e])
    nkr = np.array([[[len(send_lists[i][k][r]) for r in range(N)]
                     for k in range(K)] for i in range(B)], np.int64)
    # round up to even so all segment-aligned chunk widths are even
    # (fp32r matmul moving operands must have an even column count)
    P = nkr.max(axis=(0, 2)).astype(int)
    P = P + (P % 2)                                    # [K]
    G = int(N * (P[0] + P[1]))

    SelP = np.zeros((B, H, G), np.float32)
    for i in range(B):
        col = 0
        for k in range(K):
            for r in range(N):
                for s in send_lists[i][k][r]:
                    SelP[i, s, col] = 1.0
                    SelP[i, 64 + r, col] = 1.0
                    col += 1
                col += P[k] - nkr[i, k, r]             # zero pad
    cnt = (P[:, None] - nkr).astype(np.float32)        # [B, K, N]
    return SelP.astype(_bf), cj, cnt, P


def _trace_program(dt, nsteps, P):
    """Straight-line Bass/Tile program for one core: IPC items, nsteps RK4
    steps fully unrolled, phase-major (items interleaved within each eval
    phase so independent work pipelines across the engines).  P[k] is the
    type-packed segment width per edge type (graph-dependent)."""
    nc = bass.Bass("TRN2", target_bir_lowering=False, debug=False,
                   enable_asserts=False, num_devices=N_CORES,
                   dynamic_dma_scratch_size=2048)

    G = int(N * (P[0] + P[1]))                         # packed grid per item
    BLK = [0, int(N * P[0])]                           # block col offsets
    # per-k chunks: (col0, ncols, seg0, nsegs), segment-aligned, <=512 cols
    CHK = []
    for k in range(K):
        g = max(1, 500 // int(P[k]))
        ch = []
        s0 = 0
        while s0 < N:
            ns = min(g, N - s0)
            ch.append((s0 * int(P[k]), ns * int(P[k]), s0, ns))
            s0 += ns
        CHK.append(ch)

    NBF = IPC * G + 2 * K * H + F                      # selp + w1 + w3 bf16 cols
    NWM = K * H + F + F + IPC * N                      # w2, w3, cj, negcnt (f32r)
    NBI = K + K + 1                                    # b1, b2, b3 (f32)
    wmat_d = nc.dram_tensor("wmat", [H, NWM], F32R, kind="ExternalInput").ap()
    bias_d = nc.dram_tensor("bias", [H, NBI], F32, kind="ExternalInput").ap()
    rsmw_d = nc.dram_tensor("rsmw", [H, NBF], BF16, kind="ExternalInput").ap()
    yst_d = nc.dram_tensor("ystate", [F, IPC, N], F32, kind="ExternalInput").ap()
    out_d = nc.dram_tensor("yout", [F, nsteps, IPC, N], F32,
                           kind="ExternalOutput").ap()

    AL = mybir.AluOpType
    AF = mybir.ActivationFunctionType

    with tile.TileContext(nc) as tc, ExitStack() as ctx:
        statics = ctx.enter_context(tc.tile_pool(name="statics", bufs=1))
        state = ctx.enter_context(tc.tile_pool(name="state", bufs=1))
        hhp = ctx.enter_context(tc.tile_pool(name="hhp", bufs=3))   # [H,500]
        mmp = ctx.enter_context(tc.tile_pool(name="mmp", bufs=2))   # [H,2500]x2k
        small = ctx.enter_context(tc.tile_pool(name="small", bufs=2))
        dys = ctx.enter_context(tc.tile_pool(name="dys", bufs=1))
        wps = ctx.enter_context(tc.tile_pool(name="wps", bufs=1, space="PSUM"))
        hps = ctx.enter_context(tc.tile_pool(name="hps", bufs=4, space="PSUM"))
        mps = ctx.enter_context(tc.tile_pool(name="mps", bufs=2, space="PSUM"))
        ups = ctx.enter_context(tc.tile_pool(name="ups", bufs=1, space="PSUM"))

        wmat = statics.tile([H, NWM], F32R, name="wmat")
        biast = statics.tile([H, NBI], F32, name="biast")
        rsmw = statics.tile([H, NBF], BF16, name="rsmw")
        nc.sync.dma_start(out=wmat, in_=wmat_d)
        nc.sync.dma_start(out=biast, in_=bias_d)
        nc.sync.dma_start(out=rsmw, in_=rsmw_d)
        w2s = wmat[:, 0:K * H].rearrange("p (k h) -> p k h", k=K)
        w3s = wmat[:, K * H:K * H + F]
        cjs = wmat[0:K, K * H + F:K * H + 2 * F]           # [2, 64] f32r
        negcnt = wmat[0:K, K * H + 2 * F:K * H + 2 * F + IPC * N].rearrange(
            "p (i n) -> p i n", i=IPC)                     # [2, IPC, 50] f32r
        b1s = biast[:, 0:K]
        b2s = biast[:, K:2 * K]
        b3s = biast[0:F, 2 * K:2 * K + 1]
        rsm_all = rsmw[:, 0:IPC * G].rearrange(
            "p (i g) -> p i g", i=IPC)
        w1all = rsmw[0:F, IPC * G:IPC * G + 2 * K * H]     # [64,512] bf16
        w3bf = rsmw[:, IPC * G + 2 * K * H:]               # [128, 64] bf16
        rsms = [rsm_all[:, i] for i in range(IPC)]

        hist = state.tile([F, nsteps, IPC, N], F32, name="hist")
        ybig = state.tile([F, IPC, N], F32, name="ybig")
        nc.sync.dma_start(out=ybig, in_=yst_d)
        asts = []
        for i in range(IPC):
            a = state.tile([H, K, H], BF16, name=f"ast{i}")
            nc.vector.memset(a, 0.0)
            asts.append(a)

        # simple engine load balancer for relu/copy jobs
        load = {"v": 0.0, "a": 0.0}

        def relu_job(out_ap, in_ap, bias_ap, nelem):
            # costs calibrated from ntff profile (both ~640ns @ 500 elems)
            cv = (120 + nelem) / 0.96 + load["v"]
            ca = (260 + nelem) / 1.2 + load["a"]
            if cv <= ca:
                load["v"] = cv
                nc.vector.tensor_scalar(out_ap, in_ap, bias_ap, 0.0,
                                        op0=AL.add, op1=AL.max)
            else:
                load["a"] = ca
                nc.scalar.activation(out_ap, in_ap, AF.Relu, bias=bias_ap)

        def eval_ode(i, ysrc, dy_out):
            """dy_out[64,50] f32 = ode(ysrc[64,50] f32) for item i."""
            ybf = small.tile([F, N], BF16, tag="ybf")
            nc.vector.tensor_scalar_add(ybf, ysrc, 0.0)   # cast-copy (TS struct)
            load["v"] += (58 + N) / 0.96

            # W1 stage: [50, 512] psum = (ys0|ys1|yr0|yr1), one matmul
            wp = wps.tile([N, 4 * H], F32, tag="wp")
            nc.tensor.matmul(wp, ybf, w1all, start=True, stop=True)
            ast = asts[i]
            nc.vector.tensor_scalar_add(
                ast[0:N], wp[:, 0:2 * H].rearrange("p (k h) -> p k h", k=K), 0.0)
            load["v"] += (120 + 2 * H) / 0.96
            nc.scalar.copy(
                ast[64:64 + N], wp[:, 2 * H:4 * H].rearrange("p (k h) -> p k h", k=K))
            load["a"] += (172 + 2 * H) / 1.2

            maggs = []
            for k in range(K):
                Pk = int(P[k])
                mm = mmp.tile([H, N * Pk], BF16, tag=f"mm{k}")
                # h-matmuls run (hps bufs - 1) chunks ahead of m-matmuls so
                # the PE never sits in-queue behind an m-matmul whose hh relu
                # hasn't landed yet.
                hts = {}

                def do_h(ci):
                    c0, cl = CHK[k][ci][:2]
                    hp = hps.tile([H, 512], F32, tag="hp")
                    nc.tensor.matmul(hp[:, 0:cl], ast[:, k, :],
                                     rsms[i][:, BLK[k] + c0:BLK[k] + c0 + cl],
                                     start=True, stop=True)
                    ht = hhp.tile([H, 512], F32R, tag="hh")
                    relu_job(ht[:, 0:cl], hp[:, 0:cl], b1s[:, k:k + 1], cl)
                    hts[ci] = ht

                nch = len(CHK[k])
                for ci in range(min(3, nch)):
                    do_h(ci)
                for ci in range(nch):
                    c0, cl = CHK[k][ci][:2]
                    mp = mps.tile([H, 512], F32, tag="mp")
                    nc.tensor.matmul(mp[:, 0:cl], w2s[:, k, :],
                                     hts.pop(ci)[:, 0:cl],
                                     start=True, stop=True)
                    if ci + 3 < nch:
                        do_h(ci + 3)
                    relu_job(mm[:, c0:c0 + cl], mp[:, 0:cl],
                             b2s[:, k:k + 1], cl)
                # segmented reduce: [H, 50 r-segs, Pk] bf16 -> [H, 50] bf16
                magg = small.tile([H, N], BF16, tag=f"magg{k}")
                with nc.allow_low_precision(reason="bf16 magg; quant err ~4e-5"):
                    nc.vector.tensor_reduce(
                        magg, mm.rearrange("p (r s) -> p r s", r=N),
                        axis=mybir.AxisListType.X, op=AL.add)
                load["v"] += (58 + N * Pk) / 0.96
                maggs.append(magg)

            u = ups.tile([F, N], F32, tag="u")
            nc.tensor.matmul(u, w3bf, maggs[0], start=True, stop=False)
            nc.tensor.matmul(u, w3bf, maggs[1], start=False, stop=False)
            # subtract junk: u += cj.T @ (-cnt_i)   (rank-2 over K partitions)
            nc.tensor.matmul(u, cjs, negcnt[:, i, :],
                             start=False, stop=True)
            nc.scalar.activation(dy_out, u, AF.Tanh, bias=b3s[:, 0:1],
                                 scale=1.0 / N)
            load["a"] += (172 + N) / 1.2

        def stt(out, in0, scal, in1):
            # out = in0 * scal + in1
            nc.vector.scalar_tensor_tensor(out, in0, float(scal), in1,
                                           op0=AL.mult, op1=AL.add)

        for s in range(nsteps):
            ycur = ybig if s == 0 else hist[:, s - 1]
            dy1 = dys.tile([F, IPC, N], F32, tag="dy1")
            dy2 = dys.tile([F, IPC, N], F32, tag="dy2")
            dy3 = dys.tile([F, IPC, N], F32, tag="dy3")
            dy4 = dys.tile([F, IPC, N], F32, tag="dy4")
            ya = dys.tile([F, IPC, N], F32, tag="ya")
            ac1 = dys.tile([F, IPC, N], F32, tag="ac1")
            ac2 = dys.tile([F, IPC, N], F32, tag="ac2")

            # phase 1: dy1 = ode(y)
            for i in range(IPC):
                eval_ode(i, ycur[:, i, :], dy1[:, i, :])
                stt(ya[:, i, :], dy1[:, i, :], dt / 2, ycur[:, i, :])
            # phase 2: dy2 = ode(y + dt/2 dy1)
            for i in range(IPC):
                eval_ode(i, ya[:, i, :], dy2[:, i, :])
                stt(ya[:, i, :], dy2[:, i, :], dt / 2, ycur[:, i, :])
            stt(ac1, dy2, 2.0, dy1)                      # batched, off-path
            # phase 3: dy3 = ode(y + dt/2 dy2)
            for i in range(IPC):
                eval_ode(i, ya[:, i, :], dy3[:, i, :])
                stt(ya[:, i, :], dy3[:, i, :], float(dt), ycur[:, i, :])
            stt(ac2, dy3, 2.0, ac1)
            # phase 4: dy4 = ode(y + dt dy3); y_new per item
            for i in range(IPC):
                eval_ode(i, ya[:, i, :], dy4[:, i, :])
                ac3 = small.tile([F, N], F32, tag="ac3")
                nc.vector.tensor_tensor(ac3, dy4[:, i, :], ac2[:, i, :],
                                        op=AL.add)
                stt(hist[:, s, i, :], ac3, dt / 6, ycur[:, i, :])

        nc.sync.dma_start(out=out_d, in_=hist)

    # TRN2 codegen allows at most one sync-wait per instruction; these Bacc
    # legalization passes split excess waits onto event-semaphore instrs.
    import bass_rust
    bass_rust.move_matmul_waits_to_ldweights(nc.m)
    bass_rust.generate_event_semaphores(nc)
    return nc


def _kernel_numpy(first_point, dt, graph, W1, b1, W2, b2, W3, b3):
    """Vectorized numpy fallback implementing the exact reference math."""
    off = np.ones((N, N)) - np.eye(N)
    recv_idx, send_idx = np.where(off)      # r-major: 49 consecutive per r
    E = len(recv_idx)
    y = first_point.reshape(B, N, F).astype(np.float32)      # [B, N, F]
    sel0 = (graph == 0)[:, :, None]                          # [B, E, 1]

    W1f = W1.astype(np.float32)   # [K, 2F, H]
    W1s = np.ascontiguousarray(W1f[:, :F].transpose(1, 0, 2).reshape(F, K * H))
    W1r = np.ascontiguousarray(W1f[:, F:].transpose(1, 0, 2).reshape(F, K * H))
    W2a = W2.astype(np.float32)
    inv_n = np.float32(1.0 / N)

    def ode(yb):
        ysd = (yb.reshape(-1, F) @ W1s).reshape(-1, N, K, H)   # [B, N, K, H]
        yrc = (yb.reshape(-1, F) @ W1r).reshape(-1, N, K, H)
        h = ysd[:, send_idx] + yrc[:, recv_idx] + b1[None, None]   # [B, E, K, H]
        np.maximum(h, 0.0, out=h)
        m0 = h[:, :, 0].reshape(-1, H) @ W2a[0] + b2[0]
        m1 = h[:, :, 1].reshape(-1, H) @ W2a[1] + b2[1]
        np.maximum(m0, 0.0, out=m0)
        np.maximum(m1, 0.0, out=m1)
        msel = np.where(sel0, m0.reshape(-1, E, H), m1.reshape(-1, E, H))
        agg = msel.reshape(-1, N, N - 1, H).sum(axis=2)         # [B, N, H]
        return np.tanh((agg * inv_n).reshape(-1, H) @ W3 + b3).reshape(-1, N, F)

    outs = [y.copy()]
    for s in range(T - 1):
        k1 = ode(y)
        k2 = ode(y + (0.5 * dt) * k1)
        k3 = ode(y + (0.5 * dt) * k2)
        k4 = ode(y + dt * k3)
        y = y + (dt / 6.0) * (k1 + 2 * k2 + 2 * k3 + k4)
        outs.append(y.copy())
    pred = np.stack(outs, axis=0)                            # [T, B, N, F]
    return np.ascontiguousarray(
        pred.transpose(1, 2, 0, 3).reshape(1, B * N, T, F).astype(np.float32))


def kernel(first_point, time_steps, graph, W1, b1, W2, b2, W3, b3):
    first_point = np.asarray(first_point, dtype=np.float32)
    time_steps = np.asarray(time_steps, dtype=np.float32)
    graph = np.asarray(graph).astype(np.int64)
    W1 = np.asarray(W1, dtype=np.float32)
    b1 = np.asarray(b1, dtype=np.float32)
    W2 = np.asarray(W2, dtype=np.float32)
    b2 = np.asarray(b2, dtype=np.float32)
    W3 = np.asarray(W3, dtype=np.float32)
    b3 = np.asarray(b3, dtype=np.float32)

    dts = np.diff(time_steps.astype(np.float64))
    assert np.allclose(dts, dts.mean(), rtol=1e-4), "non-uniform dt unsupported"
    dt = float(dts.mean())

    if not _HAVE_BASS or os.environ.get("KFORCE_NUMPY", "0") == "1":
        return _kernel_numpy(first_point, dt, graph, W1, b1, W2, b2, W3, b3)

    SelP, cj, cnt, P = _build_static(graph, W1, b1, W2, b2, W3, b3)
    G = int(N * (P[0] + P[1]))

    # y0 per item, transposed: [B, F, N]
    y0t = np.ascontiguousarray(
        first_point.reshape(B, N, F).transpose(0, 2, 1)).astype(np.float32)

    w1t = np.zeros((H, 2 * K * H), np.float32)
    w1t[0:F] = W1.reshape(K, 2, F, H).transpose(2, 1, 0, 3).reshape(F, 2 * K * H)

    nsteps = int(os.environ.get("KNSTEPS", str(T - 1)))
    n_launch = -(-(T - 1) // nsteps)
    try:
        nc = _trace_program(dt, nsteps, P)

        NWM = K * H + F + F + IPC * N
        bias_h = np.zeros((H, K + K + 1), np.float32)
        bias_h[:, 0:K] = b1.T
        bias_h[:, K:2 * K] = b2.T
        bias_h[0:F, 2 * K] = b3
        wmats, rsmws = [], []
        for c in range(N_CORES):
            sl = slice(c * IPC, (c + 1) * IPC)
            wm = np.zeros((H, NWM), np.float32)
            wm[:, 0:K * H] = W2.transpose(1, 0, 2).reshape(H, K * H)
            wm[:, K * H:K * H + F] = W3
            wm[0:K, K * H + F:K * H + 2 * F] = cj
            wm[0:K, K * H + 2 * F:K * H + 2 * F + IPC * N] = (
                -cnt[sl].transpose(1, 0, 2).reshape(K, IPC * N))
            wmats.append(np.ascontiguousarray(wm))
            rsmws.append(np.ascontiguousarray(np.concatenate([
                SelP[sl].transpose(1, 0, 2).reshape(H, IPC * G)
                .astype(np.float32), w1t, W3], axis=1).astype(_bf)))

        ystate = [np.ascontiguousarray(y0t[c * IPC:(c + 1) * IPC].transpose(1, 0, 2))
                  for c in range(N_CORES)]
        chunks = []
        kernel.last_results = []
        want_trace = bool(int(os.environ.get("KTRACE", "1")))
        if want_trace:
            _install_ntff_shim()
        for L in range(n_launch):
            in_maps = [{"wmat": wmats[c], "bias": bias_h, "rsmw": rsmws[c],
                        "ystate": ystate[c]}
                       for c in range(N_CORES)]
            try:
                res = run_bass_kernel_spmd(nc, in_maps,
                                           core_ids=list(range(N_CORES)),
                                           trace=want_trace)
            except Exception:
                if not want_trace:
                    raise
                import traceback
                traceback.print_exc()
                print("kernel: traced launch failed; retrying untraced")
                want_trace = False
                res = run_bass_kernel_spmd(nc, in_maps,
                                           core_ids=list(range(N_CORES)),
                                           trace=False)
            kernel.last_results.append(res)
            outs = [r["yout"] for r in res.results]          # [F, ns, IPC, N]
            chunks.append(np.stack(outs, axis=0))            # [C, F, ns, IPC, N]
            ystate = [np.ascontiguousarray(o_[:, -1, :, :]) for o_ in outs]

        allc = np.concatenate(chunks, axis=2)[:, :, :T - 1]  # [C, F, T-1, IPC, N]
        yout = np.transpose(allc, (0, 3, 2, 1, 4)).reshape(B, T - 1, F, N)
        full = np.concatenate([y0t[:, None], yout], axis=1)   # [B, T, F, N]
        pred = np.transpose(full, (0, 3, 1, 2)).reshape(1, B * N, T, F)
        return np.ascontiguousarray(pred.astype(np.float32))
    except Exception as e:
        import traceback
        traceback.print_exc()
        print("kernel: device path failed; numpy fallback", repr(e)[:200])
        return _kernel_numpy(first_point, dt, graph, W1, b1, W2, b2, W3, b3)


if __name__ == "__main__":
    import reference
    inputs = {k: np.asarray(v) for k, v in reference.setup_inputs().items()}
    out = kernel(**inputs)
    print("out", out.shape, out.dtype)
